# revision 1
# baseline (speedup 1.0000x reference)
"""Trainium2 Bass kernel for nn_DecoderRNN (240-step LSTM decoder, B=512, H=2048).

Sharding: 8-way tensor parallel. Each core owns 1024 of the 8192 gate rows
(256 rows of each of i/f/g/o) and the matching 256 rows of h/c/z. All weights
stay SBUF-resident in bf16. Per step: two group-split AllGathers of h and two
of z (128 rows x 512 batch, bf16); fc2 is computed replicated on every core so
no AllReduce is needed. The one-hot class encoding folds into the gates matmul
as a K=40 tile; biases fold into scalar-engine activations. The emission is
software-pipelined: the next step's W_hh matmuls are emitted between fc1 and
fc2 so the tensor engine has work while the z AllGather is in flight.

Hidden-dim device order is [group, core, row]: device row p = g*1024 + j*128 + r
maps to global hidden row j*256 + g*128 + r (weights are row-permuted on host).
"""

import sys

if "/opt/trn_rl_repo" not in sys.path:
    sys.path.insert(0, "/opt/trn_rl_repo")

import numpy as np
import ml_dtypes

B = 512
OUT = 165
H = 2048
NCLS = 40
NC = 8
BL = B // NC  # batch columns stored per core
KT = H // 128  # 16 k-tiles over the hidden dim
MR = [128, OUT - 128]  # row-tile sizes for the 165-row out/fc2 dim

_CACHE = {}

# Ablation knobs (timing experiments only; ABLATE_CC breaks correctness).
ABLATE_CC = False
ABLATE_STORE = False

# device hidden row p = g*1024 + j*128 + r  <->  global row j*256 + g*128 + r
_g = np.arange(H) // 1024
_j = (np.arange(H) % 1024) // 128
_r = np.arange(H) % 128
PERM = _j * 256 + _g * 128 + _r  # device row p holds global row PERM[p]


def _build(L):
    import concourse.bacc as bacc
    import concourse.mybir as mybir
    import concourse.tile as tile
    from concourse.bass import ds
    from contextlib import ExitStack

    f32 = mybir.dt.float32
    bf16 = mybir.dt.bfloat16
    AF = mybir.ActivationFunctionType
    RG = [list(range(NC))]

    nc = bacc.Bacc("TRN2", target_bir_lowering=False, debug=False, num_devices=NC)

    whh_d = nc.dram_tensor("whh", [H, 1024], bf16, kind="ExternalInput")
    wih_d = nc.dram_tensor("wih", [OUT, 1024], bf16, kind="ExternalInput")
    moh_d = nc.dram_tensor("moh", [NCLS, 1024], bf16, kind="ExternalInput")
    wfc1_d = nc.dram_tensor("wfc1", [H, 256], bf16, kind="ExternalInput")
    wfc2_d = nc.dram_tensor("wfc2", [H, OUT], bf16, kind="ExternalInput")
    onehot_d = nc.dram_tensor("onehot", [NCLS, B], bf16, kind="ExternalInput")
    bgates_d = nc.dram_tensor("bgates", [128, 8], f32, kind="ExternalInput")
    bz_d = nc.dram_tensor("bz", [128, 2], f32, kind="ExternalInput")
    bo_d = nc.dram_tensor("bo", [128, 2], f32, kind="ExternalInput")
    h0_d = nc.dram_tensor("h0", [H, B], bf16, kind="ExternalInput")
    c0_d = nc.dram_tensor("c0", [256, B], f32, kind="ExternalInput")
    out0_d = nc.dram_tensor("out0", [OUT, B], bf16, kind="ExternalInput")
    outs_d = nc.dram_tensor("outs", [L, OUT, BL], f32, kind="ExternalOutput")

    with tile.TileContext(nc) as tc, ExitStack() as ctx:
        const = ctx.enter_context(tc.tile_pool(name="const", bufs=1))
        state = ctx.enter_context(tc.tile_pool(name="state", bufs=2))
        work = ctx.enter_context(tc.tile_pool(name="work", bufs=2))
        psum = ctx.enter_context(tc.tile_pool(name="psum", bufs=8, space="PSUM"))
        dram = ctx.enter_context(tc.tile_pool(name="dram", bufs=3, space="DRAM"))

        pid = nc.gpsimd.partition_id()
        col0 = pid * BL

        # ---- constants into SBUF
        whh_sb = const.tile([128, KT * 1024], bf16, name="whh_sb")
        nc.sync.dma_start(
            whh_sb.rearrange("p (k m) -> p k m", k=KT),
            whh_d.ap().rearrange("(k p) m -> p k m", p=128),
        )
        wih0_sb = const.tile([128, 1024], bf16, name="wih0_sb")
        nc.sync.dma_start(wih0_sb[:], wih_d.ap()[0:128, :])
        wih1_sb = const.tile([37, 1024], bf16, name="wih1_sb")
        nc.sync.dma_start(wih1_sb[:], wih_d.ap()[128:165, :])
        moh_sb = const.tile([NCLS, 1024], bf16, name="moh_sb")
        nc.sync.dma_start(moh_sb[:], moh_d.ap()[:, :])
        wfc1_sb = const.tile([128, KT * 256], bf16, name="wfc1_sb")
        nc.sync.dma_start(
            wfc1_sb.rearrange("p (k m) -> p k m", k=KT),
            wfc1_d.ap().rearrange("(k p) m -> p k m", p=128),
        )
        wfc2_sb = const.tile([128, KT * OUT], bf16, name="wfc2_sb")
        nc.sync.dma_start(
            wfc2_sb.rearrange("p (k m) -> p k m", k=KT),
            wfc2_d.ap().rearrange("(k p) m -> p k m", p=128),
        )
        onehot_sb = const.tile([NCLS, B], bf16, name="onehot_sb")
        nc.sync.dma_start(onehot_sb[:], onehot_d.ap()[:, :])
        bg_sb = const.tile([128, 8], f32, name="bg_sb")
        nc.sync.dma_start(bg_sb[:], bgates_d.ap()[:, :])
        bz_sb = const.tile([128, 2], f32, name="bz_sb")
        nc.sync.dma_start(bz_sb[:], bz_d.ap()[:, :])
        bo_sb = const.tile([128, 2], f32, name="bo_sb")
        nc.sync.dma_start(bo_sb[:], bo_d.ap()[:, :])

        def load_half(dst, src_ap, split=False, eng=None):
            # dst: SBUF [128, 8*B]; src: DRAM [1024, B] (8 row-blocks of 128)
            eng = eng or nc.sync
            if split:
                # first two k-slices land early so dependent matmuls start sooner
                eng.dma_start(
                    dst[:, :2 * B].rearrange("p (k n) -> p k n", k=2),
                    src_ap[0:256, :].rearrange("(k p) n -> p k n", p=128),
                )
                eng.dma_start(
                    dst[:, 2 * B:].rearrange("p (k n) -> p k n", k=6),
                    src_ap[256:1024, :].rearrange("(k p) n -> p k n", p=128),
                )
            else:
                eng.dma_start(
                    dst.rearrange("p (k n) -> p k n", k=8),
                    src_ap.rearrange("(k p) n -> p k n", p=128),
                )

        # ---- initial state (h0 pre-permuted on host to device order)
        hcat = []
        for g in range(2):
            hg = state.tile([128, 8 * B], bf16, tag=f"hcat{g}", name=f"hcat{g}_init")
            load_half(hg, h0_d.ap()[g * 1024:(g + 1) * 1024, :])
            hcat.append(hg)
        outb0 = state.tile([128, B], bf16, tag="outb0", name="outb0_init")
        nc.sync.dma_start(outb0[:], out0_d.ap()[0:128, :])
        outb1 = state.tile([37, B], bf16, tag="outb1", name="outb1_init")
        nc.sync.dma_start(outb1[:], out0_d.ap()[128:165, :])
        c_prev = []
        for g in range(2):
            ct = state.tile([128, B], f32, tag=f"c{g}", name=f"c{g}_init")
            nc.sync.dma_start(ct[:], c0_d.ap()[g * 128:(g + 1) * 128, :])
            c_prev.append(ct)

        def emit_whh(t, mtiles, hc):
            # gates(t) W_hh k-tiles for the given output m-tiles; returns psum tiles
            tiles = {}
            for mt in mtiles:
                ps = psum.tile([128, B], f32, tag="ps", name=f"psg_{t}_{mt}")
                tiles[mt] = ps
                for half in range(2):
                    for kk in range(8):
                        ki = half * 8 + kk
                        nc.tensor.matmul(
                            ps[:],
                            whh_sb[:, ki * 1024 + mt * 128: ki * 1024 + (mt + 1) * 128],
                            hc[half][:, kk * B:(kk + 1) * B],
                            start=(ki == 0),
                            stop=False,
                        )
            return tiles

        def ag(inp, tag, t):
            out_t = dram.tile([1024, B], bf16, tag=tag, name=f"{tag}_{t}",
                              addr_space="Shared")
            if ABLATE_CC:
                nc.sync.dma_start(out_t[0:128, :], inp[:])
            else:
                nc.gpsimd.collective_compute(
                    "AllGather", mybir.AluOpType.bypass, replica_groups=RG,
                    ins=[inp.opt()], outs=[out_t.opt()],
                )
            return out_t

        # prime: gates(0) W_hh for all 8 m-tiles
        psg = emit_whh(0, range(8), hcat)

        for t in range(L):
            # ---- gates(t) tail + LSTM, interleaved per group so the g0
            # AllGather is issued while PE still runs the g1 tail matmuls
            hb_out = [None, None]
            c_new_list = []
            for g in range(2):
                for mt in range(4 * g, 4 * g + 4):
                    ps = psg[mt]
                    nc.tensor.matmul(ps[:], wih0_sb[:, mt * 128:(mt + 1) * 128], outb0[:],
                                     start=False, stop=False)
                    nc.tensor.matmul(ps[:], wih1_sb[:, mt * 128:(mt + 1) * 128], outb1[:],
                                     start=False, stop=False)
                    nc.tensor.matmul(ps[:], moh_sb[:, mt * 128:(mt + 1) * 128], onehot_sb[:],
                                     start=False, stop=True)
                pi, pf, pg_, po = (psg[g * 4 + q] for q in range(4))
                si = work.tile([128, B], f32, tag="si", name=f"si_{t}_{g}")
                nc.scalar.activation(si[:], pi[:], AF.Sigmoid, bias=bg_sb[:, 4 * g: 4 * g + 1])
                sf = work.tile([128, B], f32, tag="sf", name=f"sf_{t}_{g}")
                nc.scalar.activation(sf[:], pf[:], AF.Sigmoid, bias=bg_sb[:, 4 * g + 1: 4 * g + 2])
                tg = work.tile([128, B], f32, tag="tg", name=f"tg_{t}_{g}")
                nc.scalar.activation(tg[:], pg_[:], AF.Tanh, bias=bg_sb[:, 4 * g + 2: 4 * g + 3])
                so = work.tile([128, B], f32, tag="so", name=f"so_{t}_{g}")
                nc.scalar.activation(so[:], po[:], AF.Sigmoid, bias=bg_sb[:, 4 * g + 3: 4 * g + 4])
                m1 = work.tile([128, B], f32, tag="m1", name=f"m1_{t}_{g}")
                nc.vector.tensor_mul(m1[:], si[:], tg[:])
                m2 = work.tile([128, B], f32, tag="m2", name=f"m2_{t}_{g}")
                nc.vector.tensor_mul(m2[:], sf[:], c_prev[g][:])
                c_new = state.tile([128, B], f32, tag=f"c{g}", name=f"c{g}_{t}")
                nc.vector.tensor_add(c_new[:], m1[:], m2[:])
                th = work.tile([128, B], f32, tag="th", name=f"th_{t}_{g}")
                nc.scalar.activation(th[:], c_new[:], AF.Tanh)
                hn = work.tile([128, B], bf16, tag="hn", name=f"hn_{t}_{g}")
                nc.vector.tensor_mul(hn[:], so[:], th[:])
                hb_in = dram.tile([128, B], bf16, tag=f"hbin{g}", name=f"hbin{g}_{t}")
                nc.sync.dma_start(hb_in[:], hn[:])
                hb_out[g] = ag(hb_in, f"hbout{g}", t)
                c_new_list.append(c_new)
            c_prev = c_new_list

            hcat_new = []
            for g in range(2):
                hg = state.tile([128, 8 * B], bf16, tag=f"hcat{g}", name=f"hcat{g}_{t}")
                load_half(hg, hb_out[g], split=(g == 0),
                          eng=(nc.sync if g == 0 else nc.scalar))
                hcat_new.append(hg)

            # ---- fc1 (own 256 rows) + relu -> z group AllGathers
            psz = []
            for mt in range(2):
                ps = psum.tile([128, B], f32, tag="ps", name=f"psz_{t}_{mt}")
                psz.append(ps)
                for half in range(2):
                    for kk in range(8):
                        ki = half * 8 + kk
                        nc.tensor.matmul(
                            ps[:],
                            wfc1_sb[:, ki * 256 + mt * 128: ki * 256 + (mt + 1) * 128],
                            hcat_new[half][:, kk * B:(kk + 1) * B],
                            start=(ki == 0),
                            stop=(ki == KT - 1),
                        )
            zb_out = [None, None]
            for mt in range(2):
                zb = work.tile([128, B], bf16, tag=f"zb{mt}", name=f"zb_{t}_{mt}")
                nc.scalar.activation(zb[:], psz[mt][:], AF.Relu, bias=bz_sb[:, mt:mt + 1])
                zb_in = dram.tile([128, B], bf16, tag=f"zbin{mt}", name=f"zbin{mt}_{t}")
                nc.sync.dma_start(zb_in[:], zb[:])
                zb_out[mt] = ag(zb_in, f"zbout{mt}", t)
            zcat = []
            for g in range(2):
                zg = state.tile([128, 8 * B], bf16, tag=f"zcat{g}", name=f"zcat{g}_{t}")
                load_half(zg, zb_out[g], eng=(nc.sync if g == 0 else nc.scalar))
                zcat.append(zg)

            # ---- prefetch next step's W_hh matmuls (m-tiles 0-5) to cover z AG
            if t + 1 < L:
                psg_next = emit_whh(t + 1, range(6), hcat_new)

            # ---- fc2 (full 165 rows, replicated) + out store + bf16 copy
            new_outb = []
            for mt in range(2):
                mr = MR[mt]
                ps = psum.tile([128, B], f32, tag="ps", name=f"pso_{t}_{mt}")
                for half in range(2):
                    for kk in range(8):
                        ki = half * 8 + kk
                        nc.tensor.matmul(
                            ps[:mr],
                            wfc2_sb[:, ki * OUT + mt * 128: ki * OUT + mt * 128 + mr],
                            zcat[half][:, kk * B:(kk + 1) * B],
                            start=(ki == 0),
                            stop=(ki == KT - 1),
                        )
                of = work.tile([128, B], f32, tag=f"of{mt}", name=f"of_{t}_{mt}")
                nc.scalar.activation(of[:mr], ps[:mr], AF.Identity, bias=bo_sb[:mr, mt:mt + 1])
                if not ABLATE_STORE:
                    nc.gpsimd.dma_start(
                        outs_d.ap()[t, mt * 128: mt * 128 + mr, :],
                        of[:mr, ds(col0, BL)],
                    )
                ob = state.tile([mr, B], bf16, tag=f"outb{mt}", name=f"outb{mt}_{t}")
                nc.vector.tensor_copy(ob[:], of[:mr])
                new_outb.append(ob)
            outb0, outb1 = new_outb

            # ---- remaining next-step W_hh m-tiles
            if t + 1 < L:
                psg_next.update(emit_whh(t + 1, range(6, 8), hcat_new))
                psg = psg_next
            hcat = hcat_new

    nc.compile()
    return nc


def _prepare_in_maps(inputs):
    bf = ml_dtypes.bfloat16
    f = {k: np.asarray(v) for k, v in inputs.items()}
    W_enc = f["W_enc"].astype(np.float32)
    b_enc = f["b_enc"].astype(np.float32)
    W_ih = f["W_ih"].astype(np.float32)
    b_ih = f["b_ih"].astype(np.float32)
    W_hh = f["W_hh"].astype(np.float32)
    b_hh = f["b_hh"].astype(np.float32)
    W_fc1 = f["W_fc1"].astype(np.float32)
    b_fc1 = f["b_fc1"].astype(np.float32)
    W_fc2 = f["W_fc2"].astype(np.float32)
    b_fc2 = f["b_fc2"].astype(np.float32)
    W_inh = f["W_inh"].astype(np.float32)
    b_inh = f["b_inh"].astype(np.float32)
    W_inc = f["W_inc"].astype(np.float32)
    b_inc = f["b_inc"].astype(np.float32)
    labels = f["labels"].astype(np.int64)
    x = f["inputs"].astype(np.float32)

    frame0 = x.reshape(B, OUT)
    h0 = frame0 @ W_inh.T + b_inh            # [B, H]
    c0 = frame0 @ W_inc.T + b_inc            # [B, H]
    onehot = np.zeros((NCLS, B), np.float32)
    onehot[labels, np.arange(B)] = 1.0
    M1 = W_ih[:, OUT:] @ W_enc               # [4H, NCLS]
    bias_gates = b_ih + b_hh + W_ih[:, OUT:] @ b_enc  # [4H]

    in_maps = []
    for j in range(NC):
        mt = np.arange(8)
        gt, g = mt % 4, mt // 4
        rows = (gt[:, None] * H + j * 256 + g[:, None] * 128 + np.arange(128)[None, :]).reshape(-1)
        zrows = j * 256 + np.arange(256)
        bg = bias_gates[rows].reshape(8, 128).T.copy()          # [128, 8]
        bzv = b_fc1[zrows].reshape(2, 128).T.copy()             # [128, 2]
        bov = np.zeros((128, 2), np.float32)
        bov[:, 0] = b_fc2[:128]
        bov[:MR[1], 1] = b_fc2[128:]
        in_maps.append({
            # k-rows over the hidden dim are permuted to device order PERM
            "whh": np.ascontiguousarray(W_hh[np.ix_(rows, PERM)].T).astype(bf),
            "wih": np.ascontiguousarray(W_ih[rows, :OUT].T).astype(bf),
            "moh": np.ascontiguousarray(M1[rows].T).astype(bf),
            "wfc1": np.ascontiguousarray(W_fc1[np.ix_(zrows, PERM)].T).astype(bf),
            "wfc2": np.ascontiguousarray(W_fc2[:, PERM].T).astype(bf),
            "onehot": onehot.astype(bf),
            "bgates": bg,
            "bz": bzv,
            "bo": bov,
            "h0": np.ascontiguousarray(h0.T[PERM]).astype(bf),
            "c0": np.ascontiguousarray(c0.T[zrows]).astype(np.float32),
            "out0": np.ascontiguousarray(frame0.T).astype(bf),
        })
    return in_maps


def _get_program(L):
    if L not in _CACHE:
        _CACHE[L] = _build(L)
    return _CACHE[L]


def kernel(**inputs):
    from concourse.bass_utils import run_bass_kernel_spmd

    L = int(np.asarray(inputs["length"]))
    x = np.asarray(inputs["inputs"])
    Bq, J, D = x.shape
    assert (Bq, J * D) == (B, OUT)

    nc = _get_program(L)
    in_maps = _prepare_in_maps(inputs)
    res = run_bass_kernel_spmd(nc, in_maps, core_ids=list(range(NC)))
    # core j returns [L, OUT, BL] covering batch columns j*BL:(j+1)*BL
    full = np.concatenate([res.results[j]["outs"] for j in range(NC)], axis=2)
    out = np.transpose(full, (2, 0, 1)).reshape(B, L, J, D).astype(np.float32)
    return out

